# revision 9
# baseline (speedup 1.0000x reference)
"""Trainium2 Bass kernel for nn_LlamaMLP (BitLinear-style ternary-quantized MLP).

Reference computation (all f32):
    s_m   = mean(|w_m|)                          (global scalar per weight)
    q_m   = round(clip(w_m / (s_m + eps), -1, 1))  (ternary)
    gate  = x @ (q_g * s_g).T ; up = x @ (q_u * s_u).T
    out   = (gate * up) @ (q_d * s_d).T
        == (s_g*s_u*s_d) * ((x @ q_g.T) * (x @ q_u.T)) @ q_d.T

Strategy: tensor-parallel over the intermediate dim I (padded to a multiple of
128*n_cores). Each core receives transposed f32 weight shards, computes a
partial |w| sum (3 scalars, AllReduce'd for the global means), quantizes its
shards to exactly-representable ternary bf16 on device, runs the three matmuls
in bf16 with f32 PSUM accumulation, and the partial [T, H] output is
ReduceScatter'd per 512-token block (pipelined behind the compute).

The host wrapper only does layout work (transpose / zero-pad / slice / concat).
"""

import sys

sys.path.insert(0, "/opt/trn_rl_repo")

import numpy as np
import concourse.mybir as mybir
import concourse.tile as tile
import concourse.bass_isa as bass_isa
from concourse import bacc
from concourse.bass_utils import run_bass_kernel_spmd

F32 = mybir.dt.float32
BF16 = mybir.dt.bfloat16
ALU = mybir.AluOpType
AX = mybir.AxisListType

P = 128
TB = 512  # token-block width (matmul moving free dim)
MAGIC = 12582912.0  # 1.5*2^23; add+sub rounds an f32 to nearest-even integer
EPS = 1e-5

# Full-problem config
FULL_T, FULL_H, FULL_I = 8192, 4096, 11008
N_CORES = 8

# Filled by kernel() when BENCH_TRACE is used via run(); read by test.py
LAST_RESULTS = None


def shard_sizes(I_real, n_cores):
    i_s = -(-I_real // (P * n_cores)) * P  # per-core padded shard (mult of 128)
    return i_s, i_s // P


def build_bass(T=FULL_T, H=FULL_H, I_real=FULL_I, n_cores=N_CORES):
    assert T % TB == 0 and H % P == 0 and H % TB == 0 and TB % n_cores == 0
    HT = H // P  # contraction tiles for gate/up
    HB = H // TB  # down-phase output column blocks
    NB = T // TB  # token blocks
    i_s, IT = shard_sizes(I_real, n_cores)
    nreal = I_real * H  # real element count of each weight matrix
    rq = TB // n_cores  # ReduceScatter rows per core per block

    nc = bacc.Bacc("TRN2", target_bir_lowering=False, debug=False, num_devices=n_cores)
    xT = nc.dram_tensor("xT", [H, T], F32, kind="ExternalInput")
    wgT = nc.dram_tensor("wgT", [H, i_s], F32, kind="ExternalInput")
    wuT = nc.dram_tensor("wuT", [H, i_s], F32, kind="ExternalInput")
    wdT = nc.dram_tensor("wdT", [i_s, H], F32, kind="ExternalInput")
    y = nc.dram_tensor("y", [NB, rq, H], F32, kind="ExternalOutput")
    rg = [list(range(n_cores))]

    with tile.TileContext(nc) as tc:
        with tc.tile_pool(name="dram", bufs=1, space="DRAM") as dram:
            # quantized weights; layouts chosen for contiguous per-partition reads
            qu_d = dram.tile([IT, P, HT * P], BF16)  # up lhsT tiles, i-major
            qd_d = dram.tile([IT, P, H], BF16)  # down rhs tiles
            # per-block output buffers: separate tiles so block b's ReduceScatter
            # doesn't serialize against block b+1's output DMAs (whole-tile deps)
            outb = [
                dram.tile([TB, H], F32, name=f"outb{b}", tag=f"outb{b}")
                for b in range(NB)
            ]
            rsb = [
                dram.tile([rq, H], F32, name=f"rsb{b}", tag=f"rsb{b}") for b in range(NB)
            ]
            cc_in = dram.tile([1, 8], F32)
            cc_out = dram.tile([1, 8], F32, addr_space="Shared")

            with tc.tile_pool(name="res", bufs=1) as rpool:
                qg_sb = rpool.tile([P, HT, i_s], BF16)  # gate lhsT, SBUF-resident
                rdenb = rpool.tile([P, 4], F32)  # 1/(s_m + eps), broadcast
                cb = rpool.tile([P, 1], F32)  # s_g*s_u*s_d, broadcast
                acc = rpool.tile([P, 4], F32)  # per-partition |w| sums
                sums = rpool.tile([1, 8], F32)
                gsums = rpool.tile([1, 8], F32)
                den = rpool.tile([1, 4], F32)
                rden = rpool.tile([1, 4], F32)
                s3 = rpool.tile([1, 4], F32)
                cprod = rpool.tile([1, 1], F32)

                srcs = [(wgT, HT, i_s), (wuT, HT, i_s), (wdT, IT, H)]

                # ---------- Phase A: global scales ----------
                with tc.tile_pool(name="scale", bufs=4) as spool:
                    nc.vector.memset(acc, 0.0)
                    for m, (w, rows, cols) in enumerate(srcs):
                        for r in range(rows):
                            st = spool.tile([P, cols], F32, tag="sst", name=f"sst{m}_{r}")
                            nc.sync.dma_start(st[:], w[r * P : (r + 1) * P, :])
                            part = spool.tile([P, 1], F32, tag="sp", name=f"sp{m}_{r}")
                            nc.vector.tensor_reduce(
                                part, st, axis=AX.X, op=ALU.add, apply_absolute_value=True
                            )
                            nc.vector.tensor_tensor(
                                acc[:, m : m + 1], acc[:, m : m + 1], part, op=ALU.add
                            )
                    nc.vector.memset(sums, 0.0)
                    for m in range(3):
                        allb = spool.tile([P, 1], F32, tag="allb", name=f"allb{m}")
                        nc.gpsimd.partition_all_reduce(
                            allb, acc[:, m : m + 1], P, bass_isa.ReduceOp.add
                        )
                        nc.vector.tensor_copy(sums[0:1, m : m + 1], allb[0:1, 0:1])
                    nc.sync.dma_start(cc_in[:], sums[:])
                    nc.gpsimd.collective_compute(
                        "AllReduce",
                        ALU.add,
                        ins=[cc_in[:]],
                        outs=[cc_out[:]],
                        replica_groups=rg,
                    )
                    nc.sync.dma_start(gsums[:], cc_out[:])
                    rn = 1.0 / float(nreal)
                    nc.vector.tensor_scalar(
                        den[0:1, 0:3], gsums[0:1, 0:3], rn, EPS, ALU.mult, ALU.add
                    )
                    nc.vector.reciprocal(rden[0:1, 0:3], den[0:1, 0:3])
                    nc.vector.tensor_scalar(
                        s3[0:1, 0:3], gsums[0:1, 0:3], rn, None, ALU.mult
                    )
                    nc.vector.tensor_tensor(cprod, s3[0:1, 0:1], s3[0:1, 1:2], op=ALU.mult)
                    nc.vector.tensor_tensor(cprod, cprod, s3[0:1, 2:3], op=ALU.mult)
                    nc.gpsimd.partition_broadcast(rdenb, rden)
                    nc.gpsimd.partition_broadcast(cb, cprod)

                # ---------- Phase B: quantize shards to ternary bf16 ----------
                # 3-engine pipeline: ACT does w*r+MAGIC (f32 add rounds to
                # nearest-even int), DVE does -MAGIC & clamp low, GpSimd does
                # clamp high + bf16 cast.
                def qround(dst, src, m, pool, cols, nm):
                    t1 = pool.tile([P, cols], F32, tag=f"qt{cols}", name=f"qt_{nm}")
                    nc.scalar.activation(
                        t1,
                        src,
                        mybir.ActivationFunctionType.Copy,
                        bias=MAGIC,
                        scale=rdenb[:, m : m + 1],
                    )
                    nc.vector.tensor_scalar(t1, t1, MAGIC, -1.0, ALU.subtract, ALU.max)
                    nc.gpsimd.tensor_scalar(dst, t1, 1.0, None, ALU.min)

                with tc.tile_pool(name="quant", bufs=3) as qpool:
                    for h in range(HT):  # gate -> SBUF resident
                        st = qpool.tile([P, i_s], F32, tag="qsg", name=f"qsg{h}")
                        nc.sync.dma_start(st[:], wgT[h * P : (h + 1) * P, :])
                        qround(qg_sb[:, h, :], st, 0, qpool, i_s, f"g{h}")
                    for h in range(HT):  # up -> DRAM [IT, P, HT*P]
                        st = qpool.tile([P, i_s], F32, tag="qsg", name=f"qsu{h}")
                        nc.sync.dma_start(st[:], wuT[h * P : (h + 1) * P, :])
                        qb = qpool.tile([P, i_s], BF16, tag="qbu", name=f"qbu{h}")
                        qround(qb, st, 1, qpool, i_s, f"u{h}")
                        for i in range(IT):
                            nc.sync.dma_start(
                                qu_d[i, :, h * P : (h + 1) * P], qb[:, i * P : (i + 1) * P]
                            )
                    CH = min(H, 2048)
                    for it in range(IT):  # down -> DRAM [IT, P, H]
                        for c0 in range(0, H, CH):
                            st = qpool.tile([P, CH], F32, tag="qsd", name=f"qsd{it}_{c0}")
                            nc.sync.dma_start(
                                st[:], wdT[it * P : (it + 1) * P, c0 : c0 + CH]
                            )
                            qb = qpool.tile([P, CH], BF16, tag="qbd", name=f"qbd{it}_{c0}")
                            qround(qb, st, 2, qpool, CH, f"d{it}_{c0}")
                            nc.sync.dma_start(qd_d[it, :, c0 : c0 + CH], qb[:])

                # ---------- Phase C: main loop over token blocks ----------
                with (
                    tc.tile_pool(name="main", bufs=2) as mpool,
                    tc.tile_pool(name="ps", bufs=8, space="PSUM") as pspool,
                ):
                    for b in range(NB):
                        xb = mpool.tile([P, HT, TB], BF16, tag="xb", bufs=1, name=f"xb{b}")
                        for h in range(HT):
                            xs = mpool.tile([P, TB], F32, tag="xs", bufs=3, name=f"xs{b}_{h}")
                            nc.sync.dma_start(
                                xs[:], xT[h * P : (h + 1) * P, b * TB : (b + 1) * TB]
                            )
                            nc.vector.tensor_copy(xb[:, h, :], xs)
                        interT = mpool.tile(
                            [P, IT, TB], BF16, tag="inter", bufs=1, name=f"int{b}"
                        )
                        for i in range(IT):
                            quc = mpool.tile(
                                [P, HT * P], BF16, tag="quc", bufs=2, name=f"quc{b}_{i}"
                            )
                            nc.sync.dma_start(quc[:], qu_d[i])
                            pg = pspool.tile([P, TB], F32, tag="ps", name=f"pg{b}_{i}")
                            for h in range(HT):
                                nc.tensor.matmul(
                                    pg,
                                    lhsT=qg_sb[:, h, i * P : (i + 1) * P],
                                    rhs=xb[:, h, :],
                                    start=(h == 0),
                                    stop=(h == HT - 1),
                                )
                            pu = pspool.tile([P, TB], F32, tag="ps", name=f"pu{b}_{i}")
                            for h in range(HT):
                                nc.tensor.matmul(
                                    pu,
                                    lhsT=quc[:, h * P : (h + 1) * P],
                                    rhs=xb[:, h, :],
                                    start=(h == 0),
                                    stop=(h == HT - 1),
                                )
                            # up PSUM -> SBUF on ACT (keeps DVE to 1 PSUM read)
                            usb = mpool.tile([P, TB], F32, tag="usb", bufs=2, name=f"usb{b}_{i}")
                            nc.scalar.activation(
                                usb, pu, mybir.ActivationFunctionType.Copy
                            )
                            nc.vector.tensor_tensor(
                                interT[:, i, :], pg, usb, op=ALU.mult
                            )
                        for hb in range(HB):
                            qdc = mpool.tile(
                                [P, IT, TB], BF16, tag="qdc", bufs=2, name=f"qdc{b}_{hb}"
                            )
                            for i in range(IT):
                                nc.sync.dma_start(
                                    qdc[:, i, :], qd_d[i, :, hb * TB : (hb + 1) * TB]
                                )
                            pos = [
                                pspool.tile([P, TB], F32, tag="ps", name=f"po{b}_{hb}_{t}")
                                for t in range(TB // P)
                            ]
                            for i in range(IT):
                                for ts in range(TB // P):
                                    nc.tensor.matmul(
                                        pos[ts],
                                        lhsT=interT[:, i, ts * P : (ts + 1) * P],
                                        rhs=qdc[:, i, :],
                                        start=(i == 0),
                                        stop=(i == IT - 1),
                                    )
                            for ts in range(TB // P):
                                ob = mpool.tile(
                                    [P, TB], F32, tag="ob", bufs=4, name=f"ob{b}_{hb}_{ts}"
                                )
                                nc.vector.tensor_scalar(ob, pos[ts], cb[:, 0:1], None, ALU.mult)
                                nc.sync.dma_start(
                                    outb[b][
                                        ts * P : (ts + 1) * P, hb * TB : (hb + 1) * TB
                                    ],
                                    ob[:],
                                )
                        # pipelined ReduceScatter of this block's partial output
                        nc.gpsimd.collective_compute(
                            "ReduceScatter",
                            ALU.add,
                            ins=[outb[b][:]],
                            outs=[rsb[b][:]],
                            replica_groups=rg,
                        )
                        nc.sync.dma_start(y[b], rsb[b][:])
    nc.compile()
    return nc


_NC_CACHE = {}


def _get_nc(T, H, I_real, n_cores):
    key = (T, H, I_real, n_cores)
    if key not in _NC_CACHE:
        _NC_CACHE[key] = build_bass(T, H, I_real, n_cores)
    return _NC_CACHE[key]


def shard_inputs(hidden_states, w_gate, w_up, w_down, n_cores=N_CORES):
    """Layout-only host prep: flatten/transpose/zero-pad/slice."""
    B, S, H = hidden_states.shape
    T = B * S
    I_real = w_gate.shape[0]
    i_s, _ = shard_sizes(I_real, n_cores)
    Ip = i_s * n_cores

    xT = np.ascontiguousarray(hidden_states.reshape(T, H).T.astype(np.float32, copy=False))
    wgT = np.zeros((H, Ip), np.float32)
    wgT[:, :I_real] = w_gate.T
    wuT = np.zeros((H, Ip), np.float32)
    wuT[:, :I_real] = w_up.T
    wdT = np.zeros((Ip, H), np.float32)
    wdT[:I_real, :] = w_down.T

    in_maps = []
    for c in range(n_cores):
        in_maps.append(
            {
                "xT": xT,
                "wgT": np.ascontiguousarray(wgT[:, c * i_s : (c + 1) * i_s]),
                "wuT": np.ascontiguousarray(wuT[:, c * i_s : (c + 1) * i_s]),
                "wdT": np.ascontiguousarray(wdT[c * i_s : (c + 1) * i_s, :]),
            }
        )
    return in_maps, (B, S, H, T)


def kernel(hidden_states, w_gate, w_up, w_down, _trace=False):
    global LAST_RESULTS
    n_cores = N_CORES
    in_maps, (B, S, H, T) = shard_inputs(hidden_states, w_gate, w_up, w_down, n_cores)
    I_real = w_gate.shape[0]
    nc = _get_nc(T, H, I_real, n_cores)
    res = run_bass_kernel_spmd(
        nc, in_maps, core_ids=list(range(n_cores)), trace=_trace
    )
    LAST_RESULTS = res

    NB = T // TB
    rq = TB // n_cores
    out = np.empty((T, H), np.float32)
    for c in range(n_cores):
        yc = res.results[c]["y"]  # [NB, rq, H]
        for b in range(NB):
            out[b * TB + c * rq : b * TB + (c + 1) * rq] = yc[b]
    return out.reshape(B, S, H)


# revision 11
# speedup vs baseline: 1.2657x; 1.2657x over previous
"""Trainium2 Bass kernel for nn_LlamaMLP (BitLinear-style ternary-quantized MLP).

Reference computation (all f32):
    s_m   = mean(|w_m|)                          (global scalar per weight)
    q_m   = round(clip(w_m / (s_m + eps), -1, 1))  (ternary)
    gate  = x @ (q_g * s_g).T ; up = x @ (q_u * s_u).T
    out   = (gate * up) @ (q_d * s_d).T
        == (s_g*s_u*s_d) * ((x @ q_g.T) * (x @ q_u.T)) @ q_d.T

Strategy: tensor-parallel over the intermediate dim I (padded to a multiple of
128*n_cores). Each core receives transposed f32 weight shards, computes a
partial |w| sum (3 scalars, AllReduce'd for the global means), quantizes its
shards to exactly-representable ternary bf16 on device, runs the three matmuls
in bf16 with f32 PSUM accumulation, and the partial [T, H] output is
ReduceScatter'd per 512-token block (pipelined behind the compute).

The host wrapper only does layout work (transpose / zero-pad / slice / concat).
"""

import sys

sys.path.insert(0, "/opt/trn_rl_repo")

import numpy as np
import concourse.mybir as mybir
import concourse.tile as tile
import concourse.bass_isa as bass_isa
from concourse import bacc
from concourse.bass_utils import run_bass_kernel_spmd

F32 = mybir.dt.float32
BF16 = mybir.dt.bfloat16
ALU = mybir.AluOpType
AX = mybir.AxisListType

P = 128
TB = 512  # token-block width (matmul moving free dim)
MAGIC = 12582912.0  # 1.5*2^23; add+sub rounds an f32 to nearest-even integer
EPS = 1e-5

# Full-problem config
FULL_T, FULL_H, FULL_I = 8192, 4096, 11008
N_CORES = 8

# Filled by kernel() when BENCH_TRACE is used via run(); read by test.py
LAST_RESULTS = None


def shard_sizes(I_real, n_cores):
    i_s = -(-I_real // (P * n_cores)) * P  # per-core padded shard (mult of 128)
    return i_s, i_s // P


def build_bass(T=FULL_T, H=FULL_H, I_real=FULL_I, n_cores=N_CORES):
    assert T % TB == 0 and H % P == 0 and H % TB == 0 and TB % n_cores == 0
    HT = H // P  # contraction tiles for gate/up
    HB = H // TB  # down-phase output column blocks
    NB = T // TB  # token blocks
    i_s, IT = shard_sizes(I_real, n_cores)
    nreal = I_real * H  # real element count of each weight matrix
    rq = TB // n_cores  # ReduceScatter rows per core per block

    nc = bacc.Bacc("TRN2", target_bir_lowering=False, debug=False, num_devices=n_cores)
    xT = nc.dram_tensor("xT", [H, T], F32, kind="ExternalInput")
    wgT = nc.dram_tensor("wgT", [H, i_s], F32, kind="ExternalInput")
    wuT = nc.dram_tensor("wuT", [H, i_s], F32, kind="ExternalInput")
    wdT = nc.dram_tensor("wdT", [i_s, H], F32, kind="ExternalInput")
    y = nc.dram_tensor("y", [NB, rq, H], F32, kind="ExternalOutput")
    rg = [list(range(n_cores))]

    with tile.TileContext(nc) as tc:
        with tc.tile_pool(name="dram", bufs=1, space="DRAM") as dram:
            # quantized weights; layouts chosen for contiguous per-partition reads
            qu_d = dram.tile([IT, P, HT * P], BF16)  # up lhsT tiles, i-major
            qd_d = dram.tile([IT, P, H], BF16)  # down rhs tiles
            # per-block output buffers: separate tiles so block b's ReduceScatter
            # doesn't serialize against block b+1's output DMAs (whole-tile deps)
            outb = [
                dram.tile([TB, H], F32, name=f"outb{b}", tag=f"outb{b}")
                for b in range(NB)
            ]
            rsb = [
                dram.tile([rq, H], F32, name=f"rsb{b}", tag=f"rsb{b}") for b in range(NB)
            ]
            cc_in = dram.tile([1, 8], F32)
            cc_out = dram.tile([1, 8], F32, addr_space="Shared")

            with tc.tile_pool(name="res", bufs=1) as rpool:
                qg_sb = rpool.tile([P, HT, i_s], BF16)  # gate lhsT, SBUF-resident
                rdenb = rpool.tile([P, 4], F32)  # 1/(s_m + eps), broadcast
                cb = rpool.tile([P, 1], F32)  # s_g*s_u*s_d, broadcast
                acc = rpool.tile([P, 4], F32)  # per-partition |w| sums
                sums = rpool.tile([1, 8], F32)
                gsums = rpool.tile([1, 8], F32)
                den = rpool.tile([1, 4], F32)
                rden = rpool.tile([1, 4], F32)
                s3 = rpool.tile([1, 4], F32)
                cprod = rpool.tile([1, 1], F32)

                srcs = [(wgT, HT, i_s), (wuT, HT, i_s), (wdT, IT, H)]

                # ---------- Phase A: global scales ----------
                with tc.tile_pool(name="scale", bufs=4) as spool:
                    nc.vector.memset(acc, 0.0)
                    for m, (w, rows, cols) in enumerate(srcs):
                        for r in range(rows):
                            st = spool.tile([P, cols], F32, tag="sst", name=f"sst{m}_{r}")
                            nc.sync.dma_start(st[:], w[r * P : (r + 1) * P, :])
                            part = spool.tile([P, 1], F32, tag="sp", name=f"sp{m}_{r}")
                            nc.vector.tensor_reduce(
                                part, st, axis=AX.X, op=ALU.add, apply_absolute_value=True
                            )
                            nc.vector.tensor_tensor(
                                acc[:, m : m + 1], acc[:, m : m + 1], part, op=ALU.add
                            )
                    nc.vector.memset(sums, 0.0)
                    for m in range(3):
                        allb = spool.tile([P, 1], F32, tag="allb", name=f"allb{m}")
                        nc.gpsimd.partition_all_reduce(
                            allb, acc[:, m : m + 1], P, bass_isa.ReduceOp.add
                        )
                        nc.vector.tensor_copy(sums[0:1, m : m + 1], allb[0:1, 0:1])
                    nc.sync.dma_start(cc_in[:], sums[:])
                    nc.gpsimd.collective_compute(
                        "AllReduce",
                        ALU.add,
                        ins=[cc_in[:]],
                        outs=[cc_out[:]],
                        replica_groups=rg,
                    )
                    nc.sync.dma_start(gsums[:], cc_out[:])
                    rn = 1.0 / float(nreal)
                    nc.vector.tensor_scalar(
                        den[0:1, 0:3], gsums[0:1, 0:3], rn, EPS, ALU.mult, ALU.add
                    )
                    nc.vector.reciprocal(rden[0:1, 0:3], den[0:1, 0:3])
                    nc.vector.tensor_scalar(
                        s3[0:1, 0:3], gsums[0:1, 0:3], rn, None, ALU.mult
                    )
                    nc.vector.tensor_tensor(cprod, s3[0:1, 0:1], s3[0:1, 1:2], op=ALU.mult)
                    nc.vector.tensor_tensor(cprod, cprod, s3[0:1, 2:3], op=ALU.mult)
                    nc.gpsimd.partition_broadcast(rdenb, rden)
                    nc.gpsimd.partition_broadcast(cb, cprod)

                # ---------- Phase B: quantize shards to ternary bf16 ----------
                # 3-engine pipeline: ACT does w*r+MAGIC (f32 add rounds to
                # nearest-even int), DVE does -MAGIC & clamp low, GpSimd does
                # clamp high + bf16 cast.
                def qround(dst, src, m, pool, cols, nm):
                    t1 = pool.tile([P, cols], F32, tag=f"qt{cols}", name=f"qt_{nm}")
                    nc.scalar.activation(
                        t1,
                        src,
                        mybir.ActivationFunctionType.Copy,
                        bias=MAGIC,
                        scale=rdenb[:, m : m + 1],
                    )
                    nc.vector.tensor_scalar(t1, t1, MAGIC, -1.0, ALU.subtract, ALU.max)
                    nc.vector.tensor_scalar(dst, t1, 1.0, None, ALU.min)

                with tc.tile_pool(name="quant", bufs=3) as qpool:
                    for h in range(HT):  # gate -> SBUF resident
                        st = qpool.tile([P, i_s], F32, tag="qsg", name=f"qsg{h}")
                        nc.sync.dma_start(st[:], wgT[h * P : (h + 1) * P, :])
                        qround(qg_sb[:, h, :], st, 0, qpool, i_s, f"g{h}")
                    for h in range(HT):  # up -> DRAM [IT, P, HT*P]
                        st = qpool.tile([P, i_s], F32, tag="qsg", name=f"qsu{h}")
                        nc.sync.dma_start(st[:], wuT[h * P : (h + 1) * P, :])
                        qb = qpool.tile([P, i_s], BF16, tag="qbu", name=f"qbu{h}")
                        qround(qb, st, 1, qpool, i_s, f"u{h}")
                        for i in range(IT):
                            nc.sync.dma_start(
                                qu_d[i, :, h * P : (h + 1) * P], qb[:, i * P : (i + 1) * P]
                            )
                    CH = min(H, 2048)
                    for it in range(IT):  # down -> DRAM [IT, P, H]
                        for c0 in range(0, H, CH):
                            st = qpool.tile([P, CH], F32, tag="qsd", name=f"qsd{it}_{c0}")
                            nc.sync.dma_start(
                                st[:], wdT[it * P : (it + 1) * P, c0 : c0 + CH]
                            )
                            qb = qpool.tile([P, CH], BF16, tag="qbd", name=f"qbd{it}_{c0}")
                            qround(qb, st, 2, qpool, CH, f"d{it}_{c0}")
                            nc.sync.dma_start(qd_d[it, :, c0 : c0 + CH], qb[:])

                # ---------- Phase C: main loop over token blocks ----------
                with (
                    tc.tile_pool(name="main", bufs=2) as mpool,
                    tc.tile_pool(name="ps", bufs=8, space="PSUM") as pspool,
                ):
                    for b in range(NB):
                        xb = mpool.tile([P, HT, TB], BF16, tag="xb", bufs=1, name=f"xb{b}")
                        for h in range(HT):
                            xs = mpool.tile([P, TB], F32, tag="xs", bufs=3, name=f"xs{b}_{h}")
                            nc.sync.dma_start(
                                xs[:], xT[h * P : (h + 1) * P, b * TB : (b + 1) * TB]
                            )
                            nc.scalar.activation(
                                xb[:, h, :], xs, mybir.ActivationFunctionType.Copy
                            )
                        interT = mpool.tile(
                            [P, IT, TB], BF16, tag="inter", bufs=1, name=f"int{b}"
                        )
                        for i in range(IT):
                            quc = mpool.tile(
                                [P, HT * P], BF16, tag="quc", bufs=2, name=f"quc{b}_{i}"
                            )
                            nc.sync.dma_start(quc[:], qu_d[i])
                            pg = pspool.tile([P, TB], F32, tag="ps", name=f"pg{b}_{i}")
                            for h in range(HT):
                                nc.tensor.matmul(
                                    pg,
                                    lhsT=qg_sb[:, h, i * P : (i + 1) * P],
                                    rhs=xb[:, h, :],
                                    start=(h == 0),
                                    stop=(h == HT - 1),
                                )
                            pu = pspool.tile([P, TB], F32, tag="ps", name=f"pu{b}_{i}")
                            for h in range(HT):
                                nc.tensor.matmul(
                                    pu,
                                    lhsT=quc[:, h * P : (h + 1) * P],
                                    rhs=xb[:, h, :],
                                    start=(h == 0),
                                    stop=(h == HT - 1),
                                )
                            # up PSUM -> SBUF on ACT (keeps DVE to 1 PSUM read)
                            usb = mpool.tile([P, TB], F32, tag="usb", bufs=2, name=f"usb{b}_{i}")
                            nc.scalar.activation(
                                usb, pu, mybir.ActivationFunctionType.Copy
                            )
                            nc.vector.tensor_tensor(
                                interT[:, i, :], pg, usb, op=ALU.mult
                            )
                        for hb in range(HB):
                            qdc = mpool.tile(
                                [P, IT, TB], BF16, tag="qdc", bufs=2, name=f"qdc{b}_{hb}"
                            )
                            for i in range(IT):
                                nc.sync.dma_start(
                                    qdc[:, i, :], qd_d[i, :, hb * TB : (hb + 1) * TB]
                                )
                            pos = [
                                pspool.tile([P, TB], F32, tag="ps", name=f"po{b}_{hb}_{t}")
                                for t in range(TB // P)
                            ]
                            for i in range(IT):
                                for ts in range(TB // P):
                                    nc.tensor.matmul(
                                        pos[ts],
                                        lhsT=interT[:, i, ts * P : (ts + 1) * P],
                                        rhs=qdc[:, i, :],
                                        start=(i == 0),
                                        stop=(i == IT - 1),
                                    )
                            for ts in range(TB // P):
                                ob = mpool.tile(
                                    [P, TB], F32, tag="ob", bufs=4, name=f"ob{b}_{hb}_{ts}"
                                )
                                nc.vector.tensor_scalar(ob, pos[ts], cb[:, 0:1], None, ALU.mult)
                                nc.sync.dma_start(
                                    outb[b][
                                        ts * P : (ts + 1) * P, hb * TB : (hb + 1) * TB
                                    ],
                                    ob[:],
                                )
                        # pipelined ReduceScatter of this block's partial output
                        nc.gpsimd.collective_compute(
                            "ReduceScatter",
                            ALU.add,
                            ins=[outb[b][:]],
                            outs=[rsb[b][:]],
                            replica_groups=rg,
                        )
                        nc.sync.dma_start(y[b], rsb[b][:])
    nc.compile()
    return nc


_NC_CACHE = {}


def _get_nc(T, H, I_real, n_cores):
    key = (T, H, I_real, n_cores)
    if key not in _NC_CACHE:
        _NC_CACHE[key] = build_bass(T, H, I_real, n_cores)
    return _NC_CACHE[key]


def shard_inputs(hidden_states, w_gate, w_up, w_down, n_cores=N_CORES):
    """Layout-only host prep: flatten/transpose/zero-pad/slice."""
    B, S, H = hidden_states.shape
    T = B * S
    I_real = w_gate.shape[0]
    i_s, _ = shard_sizes(I_real, n_cores)
    Ip = i_s * n_cores

    xT = np.ascontiguousarray(hidden_states.reshape(T, H).T.astype(np.float32, copy=False))
    wgT = np.zeros((H, Ip), np.float32)
    wgT[:, :I_real] = w_gate.T
    wuT = np.zeros((H, Ip), np.float32)
    wuT[:, :I_real] = w_up.T
    wdT = np.zeros((Ip, H), np.float32)
    wdT[:I_real, :] = w_down.T

    in_maps = []
    for c in range(n_cores):
        in_maps.append(
            {
                "xT": xT,
                "wgT": np.ascontiguousarray(wgT[:, c * i_s : (c + 1) * i_s]),
                "wuT": np.ascontiguousarray(wuT[:, c * i_s : (c + 1) * i_s]),
                "wdT": np.ascontiguousarray(wdT[c * i_s : (c + 1) * i_s, :]),
            }
        )
    return in_maps, (B, S, H, T)


def kernel(hidden_states, w_gate, w_up, w_down, _trace=False):
    global LAST_RESULTS
    n_cores = N_CORES
    in_maps, (B, S, H, T) = shard_inputs(hidden_states, w_gate, w_up, w_down, n_cores)
    I_real = w_gate.shape[0]
    nc = _get_nc(T, H, I_real, n_cores)
    res = run_bass_kernel_spmd(
        nc, in_maps, core_ids=list(range(n_cores)), trace=_trace
    )
    LAST_RESULTS = res

    NB = T // TB
    rq = TB // n_cores
    out = np.empty((T, H), np.float32)
    for c in range(n_cores):
        yc = res.results[c]["y"]  # [NB, rq, H]
        for b in range(NB):
            out[b * TB + c * rq : b * TB + (c + 1) * rq] = yc[b]
    return out.reshape(B, S, H)
